# revision 2
# baseline (speedup 1.0000x reference)
"""Trainium2 Bass kernel for nn_Cov2GEN (GNN message passing, 2x GENConv).

Sharding: 16 whole graphs per core (8 cores), each graph padded to S node
slots -> identical program structure on every core (SPMD), per-core data only.
Edges live on the core owning their dst node, sorted into 128-slot dst
windows, each window padded to CPW chunks of 128 edges.

Per conv: h[src] rows come from dma_gather; msg/exp/msg*exp computed
elementwise (DVE+ACT); segment sums (softmax denominator and weighted sum)
via one-hot selector matmuls accumulated in PSUM per window.  LSTM-cell
embeddings for nodes and edges are computed once in a single tanh-only
table phase (sigmoid(x) = 0.5 + 0.5*tanh(x/2) keeps ACT on one func set).
BatchNorm (eval) is folded into the conv MLP weights on host.

Datatypes: all big tables (h, ea), gathers, selector one-hots and the
mid AllGather are fp16 (halves DMA + 2x DVE); LSTM/selector/MLP matmuls
run fp16, remaining f32 matmuls are bitcast to f32r (1 cycle/row vs 4).
Two AllGathers: h' between convs, and per-graph outputs at the end.
"""

import math
from contextlib import ExitStack

import ml_dtypes
import numpy as np

import concourse.bacc as bacc
import concourse.bass as bass
import concourse.mybir as mybir
import concourse.tile as tile
from concourse.bass_utils import run_bass_kernel_spmd
from concourse.masks import make_identity

F32 = mybir.dt.float32
F32R = mybir.dt.float32r
F16 = mybir.dt.float16
I16 = mybir.dt.int16
I32 = mybir.dt.int32
AF = mybir.ActivationFunctionType
ALU = mybir.AluOpType
AX = mybir.AxisListType

N, E, H, G = 20000, 320000, 256, 128
FN, FE, OUT = 64, 16, 10
NCORE = 8
GPC = G // NCORE
EPS_GEN = 1e-7
EPS_BN = 1e-5
EPS_IN = 1e-5

_CACHE = {}
LAST_EXEC_NS = None


def _r(ap):
    """Reinterpret an f32 AP as f32r for full-rate PE matmul."""
    return ap.bitcast(F32R)


def _prep(inputs):
    d = {}
    x = np.asarray(inputs["x"], np.float32)
    edge_index = np.asarray(inputs["edge_index"], np.int64)
    edge_attr = np.asarray(inputs["edge_attr"], np.float32)
    batch = np.asarray(inputs["batch"], np.int64)

    cnt = np.bincount(batch, minlength=G).astype(np.int64)
    assert cnt.min() > 0
    gstart = np.concatenate([[0], np.cumsum(cnt)[:-1]])
    S = max(160, int(math.ceil(cnt.max() / 8.0)) * 8)
    NSLOT_C = GPC * S
    NW = NSLOT_C // 128
    NSLOTS = NCORE * NSLOT_C
    assert NSLOTS <= 32767

    g_of_n = batch
    r_of_n = np.arange(N) - gstart[g_of_n]
    slot_of_n = (g_of_n // GPC) * NSLOT_C + (g_of_n % GPC) * S + r_of_n
    core_of_n = g_of_n // GPC

    x_slotT = np.zeros((128, NSLOTS), np.float16)
    x_slotT[:FN, slot_of_n] = x.T
    x_slotT[FN, slot_of_n] = 1.0
    d["x_slotT"] = x_slotT

    src, dst = edge_index[0], edge_index[1]
    dst_core = core_of_n[dst]
    dst_slot_local = slot_of_n[dst] - dst_core * NSLOT_C
    win_of_e = dst_slot_local // 128

    order_cw = [[[] for _ in range(NW)] for _ in range(NCORE)]
    for e in range(E):
        order_cw[dst_core[e]][win_of_e[e]].append(e)
    maxw = max(len(l) for c in order_cw for l in c)
    CPW = int(math.ceil(math.ceil(maxw / 128.0) / 4.0)) * 4
    E_SH = NW * CPW * 128
    src_slot = slot_of_n[src]
    d.update(S=S, NSLOT_C=NSLOT_C, NW=NW, NSLOTS=NSLOTS, CPW=CPW, E_SH=E_SH)

    eaT_l, idx_l, sel_l, b2_l, icnt_l, mask_l, idxo_l = [], [], [], [], [], [], []
    for k in range(NCORE):
        order = np.full(E_SH, -1, np.int64)
        for w in range(NW):
            lst = order_cw[k][w]
            order[w * CPW * 128 : w * CPW * 128 + len(lst)] = lst
        real = order >= 0
        oe = np.where(real, order, 0)

        eaT = np.zeros((32, E_SH), np.float16)
        eaT[:FE, :] = (edge_attr[oe].T * real).astype(np.float16)
        eaT[FE, :] = real
        eaT_l.append(eaT)
        idx_l.append(np.ascontiguousarray(
            np.where(real, src_slot[oe], 0).astype(np.int32)
            .reshape(-1, 128).T))
        sel = np.zeros((E_SH, 128), np.float32)
        sel[np.arange(E_SH), np.where(real, dst_slot_local[oe] % 128, 0)] = real
        sel_l.append(sel)

        b2 = np.zeros((17, NSLOT_C), np.float32)
        mask = np.zeros(NSLOT_C, np.float32)
        for gl in range(GPC):
            gg = k * GPC + gl
            b2[gl, gl * S : gl * S + cnt[gg]] = 1.0
            mask[gl * S : gl * S + cnt[gg]] = 1.0
        b2[16] = 1.0 - mask
        b2_l.append(b2)
        ic = (1.0 / np.maximum(cnt[k * GPC : (k + 1) * GPC], 1.0)).astype(
            np.float32)
        icnt_l.append(np.tile(np.concatenate([ic, ic]), (128, 1)))
        mask_l.append(np.tile(mask, (128, 1)).astype(np.float16))
        idxo_l.append(np.ascontiguousarray(
            (k * NSLOT_C + np.arange(NSLOT_C)).astype(np.int32)
            .reshape(-1, 128).T))

    d["eaT"], d["idxs"], d["sel"], d["B2"] = eaT_l, idx_l, sel_l, b2_l
    d["icnt2"], d["mask"], d["idxown"] = icnt_l, mask_l, idxo_l

    def f32a(name):
        return np.asarray(inputs[name], np.float32)

    def ig_pack(W, b, kdim, kpad):
        Wig = np.concatenate([W[0:H], W[2 * H : 3 * H]], axis=0)
        big = np.concatenate([b[0:H], b[2 * H : 3 * H]])
        out = np.zeros((kpad, 2 * H), np.float16)
        out[:kdim] = Wig.T.astype(np.float16)
        out[kdim] = big.astype(np.float16)
        return out

    d["nodeW"] = ig_pack(f32a("node_W"), f32a("node_b"), FN, 128)
    d["edgeW"] = ig_pack(f32a("edge_W"), f32a("edge_b"), FE, 32)

    for ci, p in ((1, "g1"), (2, "g2")):
        w1, b1 = f32a(p + "_w1"), f32a(p + "_b1")
        sc = f32a(p + "_gamma") / np.sqrt(1.0 + EPS_BN)
        w1e = w1 * sc[:, None]
        b1e = b1 * sc + f32a(p + "_beta")
        w2, b2w = f32a(p + "_w2"), f32a(p + "_b2")
        w1p = np.zeros((128, 1024), np.float32)
        for cc in range(2):
            for mc in range(4):
                w1p[:, cc * 512 + mc * 128 : cc * 512 + (mc + 1) * 128] = \
                    w1e[mc * 128 : (mc + 1) * 128,
                        cc * 128 : (cc + 1) * 128].T
        w2p = np.zeros((128, 1024), np.float32)
        for kc in range(4):
            for mc in range(2):
                w2p[:, kc * 256 + mc * 128 : kc * 256 + (mc + 1) * 128] = \
                    w2[mc * 128 : (mc + 1) * 128,
                       kc * 128 : (kc + 1) * 128].T
        d[f"w1_{ci}"], d[f"w2_{ci}"] = w1p, w2p
        d[f"b1_{ci}"] = np.ascontiguousarray(b1e.reshape(4, 128).T)
        d[f"b2_{ci}"] = np.ascontiguousarray(b2w.reshape(2, 128).T)
    d["t1"] = float(np.asarray(inputs["t1"]))
    d["t2"] = float(np.asarray(inputs["t2"]))
    lwp = np.zeros((128, 2 * OUT), np.float32)
    for cc in range(2):
        lwp[:, cc * OUT : (cc + 1) * OUT] = \
            f32a("lin_w")[:, cc * 128 : (cc + 1) * 128].T
    d["linw"] = lwp
    d["linb"] = np.tile(f32a("lin_b")[None, :], (GPC, 1))
    return d


def _build(d):
    S, NSLOT_C, NW, NSLOTS, CPW, E_SH = (
        d["S"], d["NSLOT_C"], d["NW"], d["NSLOTS"], d["CPW"], d["E_SH"])
    nc = bacc.Bacc("TRN2", target_bir_lowering=False, debug=False,
                   num_devices=NCORE)

    def din(name, shape, dt=F32):
        return nc.dram_tensor(name, shape, dt, kind="ExternalInput")

    t_x = din("x_slotT", [128, NSLOTS], F16)
    t_eaT = din("eaT", [32, E_SH], F16)
    t_sel = din("sel", [E_SH, 128], F32R)
    t_idx = din("idxs", [128, E_SH // 128], I32)
    t_idxo = din("idxown", [128, NSLOT_C // 128], I32)
    t_B2 = din("B2", [17, NSLOT_C])
    t_ic2 = din("icnt2", [128, 2 * GPC])
    t_mask = din("mask", [128, NSLOT_C], F16)
    t_nodeW = din("nodeW", [128, 512], F16)
    t_edgeW = din("edgeW", [32, 512], F16)
    t_w1 = [din("w1_1", [128, 1024]), din("w1_2", [128, 1024])]
    t_b1 = [din("b1_1", [128, 4]), din("b1_2", [128, 4])]
    t_w2 = [din("w2_1", [128, 1024]), din("w2_2", [128, 1024])]
    t_b2g = [din("b2_1", [128, 2]), din("b2_2", [128, 2])]
    t_linw = din("linw", [128, 2 * OUT])
    t_linb = din("linb", [GPC, OUT])
    t_out = nc.dram_tensor("out", [G, OUT], F32, kind="ExternalOutput")

    h_hbm = nc.dram_tensor("h_tab", [NSLOTS, H], F32)
    ea_hbm = nc.dram_tensor("ea_tab", [E_SH, H], F32)
    agin = nc.dram_tensor("agin", [NSLOT_C, H], F32)
    agout = nc.dram_tensor("agout", [NSLOTS, H], F32, addr_space="Shared")
    agin2 = nc.dram_tensor("agin2", [GPC, OUT], F32)
    agout2 = nc.dram_tensor("agout2", [G, OUT], F32, addr_space="Shared")
    RG = [list(range(NCORE))]

    with tile.TileContext(nc) as tc, ExitStack() as ctx:
        cpool = ctx.enter_context(tc.tile_pool(name="const", bufs=1))
        big = ctx.enter_context(tc.tile_pool(name="big", bufs=1))
        sp = ctx.enter_context(tc.tile_pool(name="stream", bufs=2))
        sp2 = ctx.enter_context(tc.tile_pool(name="stream2", bufs=2))
        pz = ctx.enter_context(tc.tile_pool(name="pz", bufs=1, space="PSUM"))
        pwin = ctx.enter_context(tc.tile_pool(name="pwin", bufs=1, space="PSUM"))
        ptp = ctx.enter_context(tc.tile_pool(name="ptp", bufs=2, space="PSUM"))

        def cload(name, shape, tsr, dt=F32):
            t = cpool.tile(shape, dt, tag=name)
            nc.sync.dma_start(t[:], tsr.ap())
            return t

        ident = cpool.tile([128, 128], F32, tag="ident", name="ident")
        make_identity(nc, ident[:])
        ident16 = cpool.tile([128, 128], F16, tag="ident16", name="ident16")
        nc.vector.tensor_copy(ident16[:], ident[:])
        nodeW = cload("nodeW", [128, 512], t_nodeW, F16)
        edgeW = cload("edgeW", [32, 512], t_edgeW, F16)
        idx_sb = cload("idx", [128, E_SH // 128], t_idx, I32)
        idxo_sb = cload("idxo", [128, NSLOT_C // 128], t_idxo, I32)
        ic2_sb = cload("ic2", [128, 2 * GPC], t_ic2)
        w1_sb = [cload(f"w1{i}", [128, 1024], t_w1[i]) for i in range(2)]
        b1_sb = [cload(f"b1{i}", [128, 4], t_b1[i]) for i in range(2)]
        w2_sb = [cload(f"w2{i}", [128, 1024], t_w2[i]) for i in range(2)]
        b2_sb = [cload(f"b2{i}", [128, 2], t_b2g[i]) for i in range(2)]
        linw_sb = cload("linw", [128, 2 * OUT], t_linw)
        linb_sb = cload("linb", [GPC, OUT], t_linb)
        mask_sb = cload("mask", [128, NSLOT_C], t_mask, F16)
        b2sel_sb = cload("b2sel", [17, NSLOT_C], t_B2)
        epsb = cpool.tile([128, 1], F32, tag="epsb", name="epsb")
        nc.vector.memset(epsb[:], EPS_IN)
        # exp-shift for conv2: softmax ratio is invariant to a constant
        # subtraction, and |normalized h| can reach ~14, which overflows
        # fp16 exp.  exp(t*msg - 8) stays in fp16 normal range both ways.
        eshift = cpool.tile([128, 1], F32, tag="eshift", name="eshift")
        nc.vector.memset(eshift[:], -8.0)
        ezero = cpool.tile([128, 1], F32, tag="ezero", name="ezero")
        nc.vector.memset(ezero[:], 0.0)

        # ===== Phase 0: node LSTM -> h table (replicated; tanh-only ACT) ====
        NCH = NSLOTS // 128
        for g0 in range(0, NCH, 4):
            nact = min(4, NCH - g0)
            xw = sp.tile([128, 4 * 128], F16, tag="xw", name="xw")
            nc.sync.dma_start(xw[:, 0 : nact * 128],
                              t_x.ap()[:, g0 * 128 : (g0 + nact) * 128])
            zp = pz.tile([128, 4, 512], F32, tag="zp", name="zp")
            for c in range(nact):
                nc.tensor.matmul(zp[:, c, :], xw[:, c * 128 : (c + 1) * 128],
                                 nodeW[:], start=True, stop=True)
            # sigmoid(i) = 0.5 + 0.5*tanh(i/2): stays on the exp/tanh table
            sg = sp.tile([128, 4, 256], F16, tag="sg", name="sg", bufs=1)
            th = sp.tile([128, 4, 256], F16, tag="th", name="th", bufs=1)
            nc.scalar.activation(sg[:, 0:nact], zp[:, 0:nact, 0:256],
                                 AF.Sigmoid)
            nc.scalar.activation(th[:, 0:nact], zp[:, 0:nact, 256:512],
                                 AF.Tanh)
            hro = sp.tile([128, 4, 256], F32, tag="hro", name="hro", bufs=1)
            nc.vector.tensor_tensor(hro[:, 0:nact], sg[:, 0:nact],
                                    th[:, 0:nact], op=ALU.mult)
            nc.sync.dma_start(
                h_hbm.ap()[g0 * 128 : (g0 + nact) * 128, :].rearrange(
                    "(c p) h -> p c h", p=128), hro[:, 0:nact])

        # ===== Phase 0b: edge LSTM -> ea table (tanh-only ACT) =====
        for e0 in range(0, E_SH, 512):
            eat = sp.tile([32, 512], F16, tag="eat", name="eat")
            nc.sync.dma_start(eat[:], t_eaT.ap()[:, e0 : e0 + 512])
            zp = pz.tile([128, 4, 512], F32, tag="zp", name="zp")
            for c in range(4):
                nc.tensor.matmul(zp[:, c, :], eat[:, c * 128 : (c + 1) * 128],
                                 edgeW[:], start=True, stop=True)
            sg = sp.tile([128, 4, 256], F16, tag="sg", name="sg", bufs=1)
            th = sp.tile([128, 4, 256], F16, tag="th", name="th", bufs=1)
            nc.scalar.activation(sg[:], zp[:, :, 0:256], AF.Sigmoid)
            nc.scalar.activation(th[:], zp[:, :, 256:512], AF.Tanh)
            ea4 = sp.tile([128, 4, 256], F32, tag="ea4w", name="ea4w", bufs=1)
            nc.vector.tensor_tensor(ea4[:], sg[:], th[:], op=ALU.mult)
            nc.sync.dma_start(
                ea_hbm.ap()[e0 : e0 + 512, :].rearrange(
                    "(c p) h -> p c h", p=128), ea4[:])

        h2T = [big.tile([128, NSLOT_C], F32, tag=f"h2T{c}", name=f"h2T{c}")
               for c in range(2)]
        hpnm = big.tile([128, NW, 256], F32, tag="hpnm", name="hpnm")
        aggr = big.tile([128, NW, 256], F32, tag="aggr", name="aggr")
        SPW = CPW // 4

        def edge_phase(conv):
            tt = d["t1"] if conv == 0 else d["t2"]
            gtab = h_hbm if conv == 0 else agout
            for w in range(NW):
                wp = pwin.tile([128, 512], F32, tag="wp", name="wp")
                for sub in range(SPW):
                    e0 = (w * SPW + sub) * 512
                    gc = e0 // 128
                    hsrc = sp.tile([128, 4, 256], F32, tag="hsrc", name="hsrc")
                    for c in range(4):
                        nc.gpsimd.indirect_dma_start(
                            out=hsrc[:, c, :], out_offset=None, in_=gtab.ap(),
                            in_offset=bass.IndirectOffsetOnAxis(
                                ap=idx_sb[:, gc + c : gc + c + 1], axis=0))
                    ea4 = sp.tile([128, 4, 256], F32, tag="ea4", name="ea4")
                    nc.sync.dma_start(
                        ea4[:], ea_hbm.ap()[e0 : e0 + 512, :].rearrange(
                            "(c p) h -> p c h", p=128))
                    r4 = sp.tile([128, 4, 256], F32, tag="r4", name="r4")
                    nc.vector.tensor_tensor(r4[:], hsrc[:], ea4[:], op=ALU.add)
                    nc.vector.tensor_scalar(r4[:], r4[:], 0.0, EPS_GEN,
                                            op0=ALU.max, op1=ALU.add)
                    em = sp.tile([128, 4, 512], F32R, tag="em", name="em",
                                 bufs=1)
                    nc.scalar.activation(em[:, :, 0:256], r4[:], AF.Exp,
                                         scale=tt,
                                         bias=(ezero if conv == 0
                                               else eshift)[:])
                    nc.vector.tensor_tensor(em[:, :, 256:512], r4[:],
                                            em[:, :, 0:256], op=ALU.mult)
                    s4 = sp.tile([128, 4, 128], F32R, tag="s4", name="s4")
                    nc.sync.dma_start(
                        s4[:], t_sel.ap()[e0 : e0 + 512, :].rearrange(
                            "(c p) m -> p c m", p=128))
                    for c in range(4):
                        ci = sub * 4 + c
                        nc.tensor.matmul(wp[:], s4[:, c, :], em[:, c, :],
                                         start=(ci == 0), stop=(ci == CPW - 1))
                se = sp2.tile([128, 256], F32, tag="se", name="se")
                nc.vector.tensor_scalar_add(se[:], wp[:, 0:256], 1e-16)
                rs = sp2.tile([128, 256], F32, tag="rs", name="rs")
                nc.vector.reciprocal(rs[:], se[:])
                nc.vector.tensor_tensor(aggr[:, w, :], wp[:, 256:512], rs[:],
                                        op=ALU.mult)

        def node_phase(conv):
            ci = conv
            if conv == 0:
                for w in range(NW):
                    nc.gpsimd.indirect_dma_start(
                        out=hpnm[:, w, :], out_offset=None, in_=h_hbm.ap(),
                        in_offset=bass.IndirectOffsetOnAxis(
                            ap=idxo_sb[:, w : w + 1], axis=0))
            hin = aggr
            nc.vector.tensor_tensor(hin[:], hpnm[:], aggr[:], op=ALU.add)
            for t0 in range(0, NSLOT_C, 512):
                tw = min(512, NSLOT_C - t0)
                hinT = sp.tile([128, 2, 512], F32, tag="hinT", name="hinT")
                for j in range(tw // 128):
                    w = t0 // 128 + j
                    for cc in range(2):
                        tp = ptp.tile([128, 128], F32, tag="tp", name="tp")
                        nc.tensor.transpose(
                            tp[:], hin[:, w, cc * 128 : (cc + 1) * 128],
                            ident[:])
                        nc.vector.tensor_copy(
                            hinT[:, cc, j * 128 : (j + 1) * 128], tp[:])
                r1 = sp2.tile([128, 4, 512], F32, tag="r1", name="r1", bufs=1)
                for mcp in range(2):
                    m1 = pz.tile([128, 2, 512], F32, tag="zp", name="m1")
                    for half in range(2):
                        mc = mcp * 2 + half
                        for cc in range(2):
                            nc.tensor.matmul(
                                m1[:, half, 0:tw],
                                w1_sb[ci][:, cc * 512 + mc * 128 :
                                          cc * 512 + (mc + 1) * 128],
                                hinT[:, cc, 0:tw],
                                start=(cc == 0), stop=(cc == 1))
                        nc.scalar.activation(
                            r1[:, mc, 0:tw], m1[:, half, 0:tw], AF.Relu,
                            bias=b1_sb[ci][:, mc : mc + 1])
                m2 = pz.tile([128, 2, 512], F32, tag="zp", name="m2")
                for mc in range(2):
                    for kc in range(4):
                        nc.tensor.matmul(
                            m2[:, mc, 0:tw],
                            w2_sb[ci][:, kc * 256 + mc * 128 :
                                      kc * 256 + (mc + 1) * 128],
                            r1[:, kc, 0:tw], start=(kc == 0), stop=(kc == 3))
                    nc.scalar.activation(
                        h2T[mc][:, t0 : t0 + tw], m2[:, mc, 0:tw], AF.Relu,
                        bias=b2_sb[ci][:, mc : mc + 1])
            # instance norm (uniform per-graph slot ranges)
            stat = sp2.tile([128, 4 * GPC], F32, tag="stat", name="stat")
            for cc in range(2):
                for t0 in range(0, NSLOT_C, 512):
                    tw = min(512, NSLOT_C - t0)
                    nc.vector.tensor_tensor(h2T[cc][:, t0 : t0 + tw],
                                            h2T[cc][:, t0 : t0 + tw],
                                            mask_sb[:, t0 : t0 + tw],
                                            op=ALU.mult)
                sqg = sp2.tile([128, S], F32, tag="sqg", name="sqg")
                for g in range(GPC):
                    nc.vector.tensor_reduce(
                        stat[:, cc * GPC + g : cc * GPC + g + 1],
                        h2T[cc][:, g * S : (g + 1) * S], axis=AX.X, op=ALU.add)
                    nc.scalar.activation(
                        sqg[:], h2T[cc][:, g * S : (g + 1) * S], AF.Square,
                        accum_out=stat[:, (2 + cc) * GPC + g :
                                       (2 + cc) * GPC + g + 1])
            mean = sp2.tile([128, 2 * GPC], F32, tag="mean", name="mean")
            nc.vector.tensor_tensor(mean[:], stat[:, 0 : 2 * GPC], ic2_sb[:],
                                    op=ALU.mult)
            var = sp2.tile([128, 2 * GPC], F32, tag="var", name="var")
            nc.vector.tensor_tensor(var[:], stat[:, 2 * GPC : 4 * GPC],
                                    ic2_sb[:], op=ALU.mult)
            msq = sp2.tile([128, 2 * GPC], F32, tag="msq", name="msq")
            nc.vector.tensor_tensor(msq[:], mean[:], mean[:], op=ALU.mult)
            nc.vector.tensor_tensor(var[:], var[:], msq[:], op=ALU.subtract)
            sd = sp2.tile([128, 2 * GPC], F32, tag="sd", name="sd")
            nc.scalar.activation(sd[:], var[:], AF.Sqrt, bias=epsb[:])
            rstd = sp2.tile([128, 2 * GPC], F32, tag="rstd", name="rstd")
            nc.vector.reciprocal(rstd[:], sd[:])
            mr = sp2.tile([128, 2 * GPC], F32, tag="mr", name="mr")
            nc.vector.tensor_tensor(mr[:], mean[:], rstd[:], op=ALU.mult)
            outT = h2T
            for cc in range(2):
                mrT = sp2.tile([17, 128], F32, tag="mrT", name="mrT")
                rsT = sp2.tile([17, 128], F32, tag="rsT", name="rsT")
                nc.vector.memset(rsT[:], 0.0)
                nc.vector.memset(mrT[:], 0.0 if conv == 0 else 1e30)
                tpa = ptp.tile([128, 128], F32, tag="tp", name="tp")
                nc.tensor.transpose(tpa[0:GPC, :],
                                    mr[:, cc * GPC : (cc + 1) * GPC],
                                    ident[:])
                nc.vector.tensor_copy(mrT[0:GPC, :], tpa[0:GPC, :])
                tpb = ptp.tile([128, 128], F32, tag="tp", name="tp")
                nc.tensor.transpose(tpb[0:GPC, :],
                                    rstd[:, cc * GPC : (cc + 1) * GPC],
                                    ident[:])
                nc.vector.tensor_copy(rsT[0:GPC, :], tpb[0:GPC, :])
                for t0 in range(0, NSLOT_C, 512):
                    tw = min(512, NSLOT_C - t0)
                    ex = pz.tile([128, 2, 512], F32, tag="zp", name="ex")
                    nc.tensor.matmul(ex[:, 0, 0:tw], rsT[:],
                                     b2sel_sb[:, t0 : t0 + tw],
                                     start=True, stop=True)
                    nc.tensor.matmul(ex[:, 1, 0:tw], mrT[:],
                                     b2sel_sb[:, t0 : t0 + tw],
                                     start=True, stop=True)
                    nc.vector.tensor_tensor(
                        outT[cc][:, t0 : t0 + tw], h2T[cc][:, t0 : t0 + tw],
                        ex[:, 0, 0:tw], op=ALU.mult)
                    nc.vector.tensor_tensor(
                        outT[cc][:, t0 : t0 + tw], outT[cc][:, t0 : t0 + tw],
                        ex[:, 1, 0:tw], op=ALU.subtract)

        edge_phase(0)
        node_phase(0)
        for w in range(NW):
            for cc in range(2):
                tp = ptp.tile([128, 128], F32, tag="tp", name="tp")
                nc.tensor.transpose(tp[:],
                                    h2T[cc][:, w * 128 : (w + 1) * 128],
                                    ident[:])
                nc.vector.tensor_copy(hpnm[:, w, cc * 128 : (cc + 1) * 128],
                                      tp[:])
        nc.sync.dma_start(
            agin.ap().rearrange("(w p) h -> p w h", p=128), hpnm[:])
        nc.gpsimd.collective_compute(
            "AllGather", ALU.bypass, replica_groups=RG,
            ins=[agin.ap().opt()], outs=[agout.ap().opt()])

        edge_phase(1)
        node_phase(1)
        pooled = sp2.tile([128, 2, GPC], F32, tag="pooled", name="pooled")
        for cc in range(2):
            for g in range(GPC):
                nc.vector.tensor_reduce(
                    pooled[:, cc, g : g + 1], h2T[cc][:, g * S : (g + 1) * S],
                    axis=AX.X, op=ALU.max)
        fo = pwin.tile([GPC, OUT], F32, tag="fo", name="fo")
        for cc in range(2):
            nc.tensor.matmul(fo[:], pooled[:, cc, :],
                             linw_sb[:, cc * OUT : (cc + 1) * OUT],
                             start=(cc == 0), stop=(cc == 1))
        fs = sp2.tile([GPC, OUT], F32, tag="fs", name="fs")
        nc.vector.tensor_tensor(fs[:], fo[:], linb_sb[:], op=ALU.add)
        fsig = sp2.tile([GPC, OUT], F32, tag="fsig", name="fsig")
        nc.scalar.activation(fsig[:], fs[:], AF.Sigmoid)
        nc.sync.dma_start(agin2.ap(), fsig[:])
        nc.gpsimd.collective_compute(
            "AllGather", ALU.bypass, replica_groups=RG,
            ins=[agin2.ap().opt()], outs=[agout2.ap().opt()])
        ofin = sp2.tile([G, OUT], F32, tag="ofin", name="ofin")
        nc.sync.dma_start(ofin[:], agout2.ap())
        nc.sync.dma_start(t_out.ap(), ofin[:])

    nc.compile()
    return nc


def _in_maps(d):
    in_maps = []
    for k in range(NCORE):
        in_maps.append(dict(
            x_slotT=d["x_slotT"], eaT=d["eaT"][k], sel=d["sel"][k],
            idxs=d["idxs"][k], idxown=d["idxown"][k], B2=d["B2"][k],
            icnt2=d["icnt2"][k], mask=d["mask"][k], nodeW=d["nodeW"],
            edgeW=d["edgeW"], w1_1=d["w1_1"], b1_1=d["b1_1"], w2_1=d["w2_1"],
            b2_1=d["b2_1"], w1_2=d["w1_2"], b1_2=d["b1_2"], w2_2=d["w2_2"],
            b2_2=d["b2_2"], linw=d["linw"], linb=d["linb"]))
    return in_maps


LAST_RES = None


def kernel(**inputs):
    global LAST_EXEC_NS, LAST_RES
    d = _prep(inputs)
    key = (d["S"], d["CPW"])
    if key not in _CACHE:
        _CACHE[key] = _build(d)
    nc = _CACHE[key]
    res = run_bass_kernel_spmd(nc, _in_maps(d), core_ids=list(range(NCORE)))
    LAST_EXEC_NS = res.exec_time_ns
    LAST_RES = res
    return res.results[0]["out"]



# revision 24
# speedup vs baseline: 1.7835x; 1.7835x over previous
"""Trainium2 Bass kernel for nn_Cov2GEN (GNN message passing, 2x GENConv).

Sharding: 16 whole graphs per core (8 cores), each graph padded to S node
slots -> identical program structure on every core (SPMD), per-core data only.
Edges live on the core owning their dst node, sorted into 128-slot dst
windows; within-graph node->slot assignment is permuted on host to balance
per-window edge counts (CPW 20 -> 16).

All big tables (h, ea), gathers, selectors, the MLP weights and the
AllGathers are fp16; accumulations stay fp32 in PSUM.  Per conv: h[src]
rows come from batched (8-row) indirect DMAs; msg/exp/msg*exp elementwise
on DVE+ACT in fp16; segment sums via one-hot selector matmuls accumulated
in PSUM per window.  Node LSTM is computed sharded (own slots only) and
replicated via an AllGather that overlaps the edge-LSTM table build.
Sigmoid is computed as 0.5 + 0.5*tanh(x/2) so the whole kernel (except the
final output sigmoid and the per-conv Sqrt) stays on one ACT table set.
BatchNorm (eval) is folded into the conv MLP weights on host; instance
norm is applied with per-graph fused tensor_scalar (mult+subtract).
"""

import math
import os
from contextlib import ExitStack

import numpy as np

import concourse.bacc as bacc
import concourse.bass as bass
import concourse.mybir as mybir
import concourse.tile as tile
from concourse.bass_utils import run_bass_kernel_spmd

F32 = mybir.dt.float32
F32R = mybir.dt.float32r
F16 = mybir.dt.float16
I32 = mybir.dt.int32
AF = mybir.ActivationFunctionType
ALU = mybir.AluOpType
AX = mybir.AxisListType

N, E, H, G = 20000, 320000, 256, 128
FN, FE, OUT = 64, 16, 10
NCORE = 8
GPC = G // NCORE
EPS_BN = 1e-5
EPS_IN = 1e-5

_CACHE = {}
LAST_EXEC_NS = None
LAST_RES = None

GB = int(os.environ.get("KGB", "8"))  # gather batch (rows per indirect DMA)


def _prep(inputs):
    d = {}
    x = np.asarray(inputs["x"], np.float32)
    edge_index = np.asarray(inputs["edge_index"], np.int64)
    edge_attr = np.asarray(inputs["edge_attr"], np.float32)
    batch = np.asarray(inputs["batch"], np.int64)

    cnt = np.bincount(batch, minlength=G).astype(np.int64)
    assert cnt.min() > 0
    gstart = np.concatenate([[0], np.cumsum(cnt)[:-1]])
    S = max(160, int(math.ceil(cnt.max() / 8.0)) * 8)
    NSLOT_C = GPC * S
    NW = NSLOT_C // 128
    NSLOTS = NCORE * NSLOT_C
    assert NSLOTS <= 32767 and NSLOT_C % 128 == 0

    src, dst = edge_index[0], edge_index[1]
    deg = np.bincount(dst, minlength=N).astype(np.int64)

    # --- balanced within-graph slot assignment -------------------------
    # Greedy: heaviest-degree nodes to the least-loaded window their graph
    # spans (pads fill leftover slots), then a local-search refinement that
    # swaps slots within a graph to push the max window load under TARGET.
    TARGET = 16 * 128 - 16
    slot_of_n = np.empty(N, np.int64)
    for k in range(NCORE):
        wload = np.zeros(NW, np.int64)
        slot_node = np.full(NSLOT_C, -1, np.int64)
        for gl in range(GPC):
            g = k * GPC + gl
            nodes = np.arange(gstart[g], gstart[g] + cnt[g])
            nd = nodes[np.argsort(-deg[nodes], kind="stable")]
            slots = np.arange(gl * S, (gl + 1) * S)
            wins = np.unique(slots // 128)
            wslots = {w: list(slots[slots // 128 == w]) for w in wins}
            for n_i in nd:
                w = min(wslots, key=lambda ww: wload[ww])
                s = wslots[w].pop()
                if not wslots[w]:
                    del wslots[w]
                slot_node[s] = n_i
                wload[w] += deg[n_i]
        sdeg = np.where(slot_node >= 0, deg[np.maximum(slot_node, 0)], 0)
        for _ in range(800):
            w = int(wload.argmax())
            if wload[w] <= TARGET:
                break
            sl = np.arange(w * 128, (w + 1) * 128)
            moved = False
            for s in sl[np.argsort(-sdeg[sl])]:
                if sdeg[s] == 0:
                    break
                gl = s // S
                cand = np.arange(gl * S, (gl + 1) * S)
                cand = cand[(cand // 128 != w) & (sdeg[cand] < sdeg[s])]
                if len(cand) == 0:
                    continue
                nm = wload[cand // 128] + sdeg[s] - sdeg[cand]
                j = int(nm.argmin())
                if nm[j] < wload[w]:
                    s2 = cand[j]
                    w2 = s2 // 128
                    d_ = sdeg[s] - sdeg[s2]
                    wload[w] -= d_
                    wload[w2] += d_
                    slot_node[s], slot_node[s2] = slot_node[s2], slot_node[s]
                    sdeg[s], sdeg[s2] = sdeg[s2], sdeg[s]
                    moved = True
                    break
            if not moved:
                break
        real = slot_node >= 0
        slot_of_n[slot_node[real]] = k * NSLOT_C + np.where(real)[0]

    g_of_n = batch
    core_of_n = g_of_n // GPC
    dst_core = core_of_n[dst]
    dst_slot_local = slot_of_n[dst] - dst_core * NSLOT_C
    win_of_e = dst_slot_local // 128

    order_cw = [[[] for _ in range(NW)] for _ in range(NCORE)]
    for e in range(E):
        order_cw[dst_core[e]][win_of_e[e]].append(e)
    maxw = max(len(l) for c in order_cw for l in c)
    CPW = max(GB, int(math.ceil(math.ceil(maxw / 128.0) / GB)) * GB)
    E_SH = NW * CPW * 128
    src_slot = slot_of_n[src]
    d.update(S=S, NSLOT_C=NSLOT_C, NW=NW, NSLOTS=NSLOTS, CPW=CPW, E_SH=E_SH)

    xl_l, eaT_l, idx_l, sel_l = [], [], [], []
    icnt_l, mask_l, padneg_l = [], [], []
    for k in range(NCORE):
        order = np.full(E_SH, -1, np.int64)
        for w in range(NW):
            lst = order_cw[k][w]
            order[w * CPW * 128 : w * CPW * 128 + len(lst)] = lst
        real = order >= 0
        oe = np.where(real, order, 0)

        eaT = np.zeros((32, E_SH), np.float16)
        eaT[:FE, :] = (edge_attr[oe].T * real).astype(np.float16)
        eaT[FE, :] = real
        eaT_l.append(eaT)
        idx_l.append(np.ascontiguousarray(
            np.where(real, src_slot[oe], 0).astype(np.int32)
            .reshape(-1, 128).T))
        sel = np.zeros((E_SH, 128), np.float32)
        sel[np.arange(E_SH), np.where(real, dst_slot_local[oe] % 128, 0)] = real
        sel_l.append(sel)

        xl = np.zeros((128, NSLOT_C), np.float16)
        nk = np.where(core_of_n == k)[0]
        loc = slot_of_n[nk] - k * NSLOT_C
        xl[:FN, loc] = x[nk].T
        xl[FN, loc] = 1.0
        xl_l.append(xl)

        mask = np.zeros(NSLOT_C, np.float32)
        mask[loc] = 1.0
        ic = (1.0 / np.maximum(cnt[k * GPC : (k + 1) * GPC], 1.0)).astype(
            np.float32)
        icnt_l.append(np.tile(np.concatenate([ic, ic]), (128, 1)))
        mask_l.append(np.tile(mask, (128, 1)).astype(np.float16))
        padneg_l.append(np.tile((mask - 1.0) * 30000.0, (128, 1))
                        .astype(np.float16))

    d["x_loc"], d["eaT"], d["idxs"], d["sel"] = xl_l, eaT_l, idx_l, sel_l
    d["icnt2"], d["mask"], d["padneg"] = icnt_l, mask_l, padneg_l

    def f32a(name):
        return np.asarray(inputs[name], np.float32)

    def ig_pack(W, b, kdim, kpad):
        Wig = np.concatenate([W[0:H], W[2 * H : 3 * H]], axis=0)
        big = np.concatenate([b[0:H], b[2 * H : 3 * H]])
        out = np.zeros((kpad, 2 * H), np.float16)
        out[:kdim] = Wig.T.astype(np.float16)
        out[kdim] = big.astype(np.float16)
        return out

    d["nodeW"] = ig_pack(f32a("node_W"), f32a("node_b"), FN, 128)
    d["edgeW"] = ig_pack(f32a("edge_W"), f32a("edge_b"), FE, 32)

    for ci, p in ((1, "g1"), (2, "g2")):
        w1, b1 = f32a(p + "_w1"), f32a(p + "_b1")
        sc = f32a(p + "_gamma") / np.sqrt(1.0 + EPS_BN)
        w1e = w1 * sc[:, None]
        b1e = b1 * sc + f32a(p + "_beta")
        w2, b2w = f32a(p + "_w2"), f32a(p + "_b2")
        w1p = np.zeros((128, 1024), np.float16)
        for cc in range(2):
            for mc in range(4):
                w1p[:, cc * 512 + mc * 128 : cc * 512 + (mc + 1) * 128] = \
                    w1e[mc * 128 : (mc + 1) * 128,
                        cc * 128 : (cc + 1) * 128].T
        w2p = np.zeros((128, 1024), np.float16)
        for kc in range(4):
            for mc in range(2):
                w2p[:, kc * 256 + mc * 128 : kc * 256 + (mc + 1) * 128] = \
                    w2[mc * 128 : (mc + 1) * 128,
                       kc * 128 : (kc + 1) * 128].T
        d[f"w1_{ci}"], d[f"w2_{ci}"] = w1p, w2p
        d[f"b1_{ci}"] = np.ascontiguousarray(b1e.reshape(4, 128).T)
        d[f"b2_{ci}"] = np.ascontiguousarray(b2w.reshape(2, 128).T)
    d["t1"] = float(np.asarray(inputs["t1"]))
    d["t2"] = float(np.asarray(inputs["t2"]))
    lwp = np.zeros((128, 2 * OUT), np.float32)
    for cc in range(2):
        lwp[:, cc * OUT : (cc + 1) * OUT] = \
            f32a("lin_w")[:, cc * 128 : (cc + 1) * 128].T
    d["linw"] = lwp
    d["linb"] = np.tile(f32a("lin_b")[None, :], (GPC, 1))
    return d


def _build(d):
    S, NSLOT_C, NW, NSLOTS, CPW, E_SH = (
        d["S"], d["NSLOT_C"], d["NW"], d["NSLOTS"], d["CPW"], d["E_SH"])
    nc = bacc.Bacc("TRN2", target_bir_lowering=False, debug=False,
                   num_devices=NCORE)

    def din(name, shape, dt=F32):
        return nc.dram_tensor(name, shape, dt, kind="ExternalInput")

    t_xloc = din("x_loc", [128, NSLOT_C], F16)
    t_eaT = din("eaT", [32, E_SH], F16)
    t_sel = din("sel", [E_SH, 128], F32R)
    t_idx = din("idxs", [128, E_SH // 128], I32)
    t_ic2 = din("icnt2", [128, 2 * GPC])
    t_mask = din("mask", [128, NSLOT_C], F16)
    t_padneg = din("padneg", [128, NSLOT_C], F16)
    t_nodeW = din("nodeW", [128, 512], F16)
    t_edgeW = din("edgeW", [32, 512], F16)
    t_w1 = [din("w1_1", [128, 1024], F16), din("w1_2", [128, 1024], F16)]
    t_b1 = [din("b1_1", [128, 4]), din("b1_2", [128, 4])]
    t_w2 = [din("w2_1", [128, 1024], F16), din("w2_2", [128, 1024], F16)]
    t_b2g = [din("b2_1", [128, 2]), din("b2_2", [128, 2])]
    t_linw = din("linw", [128, 2 * OUT])
    t_linb = din("linb", [GPC, OUT])
    t_out = nc.dram_tensor("out", [G, OUT], F32, kind="ExternalOutput")

    ea_hbm = nc.dram_tensor("ea_tab", [E_SH, H], F16)
    agin_h = nc.dram_tensor("agin_h", [NSLOT_C, H], F16)
    agout_h = nc.dram_tensor("agout_h", [NSLOTS, H], F16, addr_space="Shared")
    # conv2's h table stays fp32: after instance norm |h| reaches ~14, and
    # fp16 quantization there becomes ~7e-3 absolute error on the exp
    # argument -> ~1e-2 relative on softmax weights (blows the 2e-2 budget)
    agin2 = nc.dram_tensor("agin2", [NSLOT_C, H], F32)
    agout2 = nc.dram_tensor("agout2", [NSLOTS, H], F32, addr_space="Shared")
    agin3 = nc.dram_tensor("agin3", [GPC, OUT], F32)
    agout3 = nc.dram_tensor("agout3", [G, OUT], F32, addr_space="Shared")
    RG = [list(range(NCORE))]
    SPW = CPW // GB

    with tile.TileContext(nc) as tc, ExitStack() as ctx:
        cpool = ctx.enter_context(tc.tile_pool(name="const", bufs=1))
        big = ctx.enter_context(tc.tile_pool(name="big", bufs=1))
        sp = ctx.enter_context(tc.tile_pool(name="stream", bufs=2))
        spg = ctx.enter_context(tc.tile_pool(name="gath", bufs=4))
        sp2 = ctx.enter_context(tc.tile_pool(name="stream2", bufs=2))
        pz = ctx.enter_context(tc.tile_pool(name="pz", bufs=1, space="PSUM"))
        pwin = ctx.enter_context(tc.tile_pool(name="pwin", bufs=1, space="PSUM"))
        ptp = ctx.enter_context(tc.tile_pool(name="ptp", bufs=1, space="PSUM"))

        def cload(name, shape, tsr, dt=F32):
            t = cpool.tile(shape, dt, tag=name)
            nc.sync.dma_start(t[:], tsr.ap())
            return t

        from concourse.masks import make_identity
        ident = cpool.tile([128, 128], F32, tag="ident", name="ident")
        make_identity(nc, ident[:])
        ident16 = cpool.tile([128, 128], F16, tag="ident16", name="ident16")
        nc.vector.tensor_copy(ident16[:], ident[:])
        nodeW = cload("nodeW", [128, 512], t_nodeW, F16)
        edgeW = cload("edgeW", [32, 512], t_edgeW, F16)
        idx_sb = cload("idx", [128, E_SH // 128], t_idx, I32)
        ic2_sb = cload("ic2", [128, 2 * GPC], t_ic2)
        w1_sb = [cload(f"w1{i}", [128, 1024], t_w1[i], F16) for i in range(2)]
        b1_sb = [cload(f"b1{i}", [128, 4], t_b1[i]) for i in range(2)]
        w2_sb = [cload(f"w2{i}", [128, 1024], t_w2[i], F16) for i in range(2)]
        b2_sb = [cload(f"b2{i}", [128, 2], t_b2g[i]) for i in range(2)]
        linw_sb = cload("linw", [128, 2 * OUT], t_linw)
        linb_sb = cload("linb", [GPC, OUT], t_linb)
        mask_sb = cload("mask", [128, NSLOT_C], t_mask, F16)
        padneg_sb = cload("padneg", [128, NSLOT_C], t_padneg, F16)
        epsb = cpool.tile([128, 1], F32, tag="epsb", name="epsb")
        nc.vector.memset(epsb[:], EPS_IN)
        # exp-shift for conv2: softmax ratio is invariant to a constant
        # subtraction; |normalized h| can reach ~14 which overflows fp16 exp.
        eshift = cpool.tile([128, 1], F32, tag="eshift", name="eshift")
        nc.vector.memset(eshift[:], -8.0)
        ezero = cpool.tile([128, 1], F32, tag="ezero", name="ezero")
        nc.vector.memset(ezero[:], 0.0)

        hpnm = big.tile([128, NW, 256], F32, tag="hpnm", name="hpnm")
        aggr = big.tile([128, NW, 256], F32, tag="aggr", name="aggr")
        h2T = [big.tile([128, NSLOT_C], F32, tag=f"h2T{c}", name=f"h2T{c}")
               for c in range(2)]

        # ===== Phase A: node LSTM for OWN slots only (tanh-only ACT) =====
        for g0 in range(0, NW, 4):
            nact = min(4, NW - g0)
            xw = sp.tile([128, 4 * 128], F16, tag="xw", name="xw")
            nc.sync.dma_start(xw[:, 0 : nact * 128],
                              t_xloc.ap()[:, g0 * 128 : (g0 + nact) * 128])
            zp = pz.tile([128, 4, 512], F32, tag="zp", name="zp")
            for c in range(nact):
                nc.tensor.matmul(zp[:, c, :], xw[:, c * 128 : (c + 1) * 128],
                                 nodeW[:], start=True, stop=True)
            # sigmoid(i) = 0.5 + 0.5*tanh(i/2): stays on the exp/tanh table
            thi = sp.tile([128, 4, 256], F16, tag="thi", name="thi", bufs=1)
            thg = sp.tile([128, 4, 256], F16, tag="thg", name="thg", bufs=1)
            nc.scalar.activation(thi[:, 0:nact], zp[:, 0:nact, 0:256],
                                 AF.Tanh, scale=0.5)
            nc.scalar.activation(thg[:, 0:nact], zp[:, 0:nact, 256:512],
                                 AF.Tanh)
            sg = sp.tile([128, 4, 256], F16, tag="sg", name="sg", bufs=1)
            nc.vector.tensor_scalar(sg[:, 0:nact], thi[:, 0:nact], 0.5, 0.5,
                                    op0=ALU.mult, op1=ALU.add)
            hro = sp.tile([128, 4, 256], F16, tag="hro", name="hro", bufs=1)
            nc.vector.tensor_tensor(hro[:, 0:nact], sg[:, 0:nact],
                                    thg[:, 0:nact], op=ALU.mult)
            nc.vector.tensor_copy(hpnm[:, g0 : g0 + nact, :], hro[:, 0:nact])
            nc.sync.dma_start(
                agin_h.ap()[g0 * 128 : (g0 + nact) * 128, :].rearrange(
                    "(c p) h -> p c h", p=128), hro[:, 0:nact])
        nc.gpsimd.collective_compute(
            "AllGather", ALU.bypass, replica_groups=RG,
            ins=[agin_h.ap().opt()], outs=[agout_h.ap().opt()])

        # ===== Phase B: edge LSTM -> ea table fp16 (tanh-only ACT) =====
        for e0 in range(0, E_SH, 512):
            eat = sp.tile([32, 512], F16, tag="eat", name="eat")
            nc.sync.dma_start(eat[:], t_eaT.ap()[:, e0 : e0 + 512])
            zp = pz.tile([128, 4, 512], F32, tag="zp", name="zp")
            for c in range(4):
                nc.tensor.matmul(zp[:, c, :], eat[:, c * 128 : (c + 1) * 128],
                                 edgeW[:], start=True, stop=True)
            thi = sp.tile([128, 4, 256], F16, tag="thi", name="thi", bufs=1)
            thg = sp.tile([128, 4, 256], F16, tag="thg", name="thg", bufs=1)
            nc.scalar.activation(thi[:], zp[:, :, 0:256], AF.Tanh, scale=0.5)
            nc.scalar.activation(thg[:], zp[:, :, 256:512], AF.Tanh)
            sg = sp.tile([128, 4, 256], F16, tag="sg", name="sg", bufs=1)
            nc.vector.tensor_scalar(sg[:], thi[:], 0.5, 0.5,
                                    op0=ALU.mult, op1=ALU.add)
            ea4 = sp.tile([128, 4, 256], F16, tag="ea4w", name="ea4w", bufs=1)
            nc.vector.tensor_tensor(ea4[:], sg[:], thg[:], op=ALU.mult)
            nc.sync.dma_start(
                ea_hbm.ap()[e0 : e0 + 512, :].rearrange(
                    "(c p) h -> p c h", p=128), ea4[:])

        def edge_phase(conv):
            tt = d["t1"] if conv == 0 else d["t2"]
            gtab = agout_h if conv == 0 else agout2
            ebias = ezero if conv == 0 else eshift
            dt_h = F16 if conv == 0 else F32
            for w in range(NW):
                wp = pwin.tile([128, 512], F32, tag="wp", name="wp")
                for sub in range(SPW):
                    e0 = (w * SPW + sub) * GB * 128
                    gc = e0 // 128
                    hsrc = spg.tile([128, GB, 256], dt_h, tag=f"hsrc{conv}",
                                    name="hsrc", bufs=(4 if conv == 0 else 3))
                    for c in range(GB):
                        # one row per partition per instruction: batched
                        # (multi-row) indirect DMAs complete their semaphore
                        # before all rows land (observed ~1.4% stale rows)
                        nc.gpsimd.indirect_dma_start(
                            out=hsrc[:, c, :], out_offset=None, in_=gtab.ap(),
                            in_offset=bass.IndirectOffsetOnAxis(
                                ap=idx_sb[:, gc + c : gc + c + 1], axis=0))
                    ea4 = sp.tile([128, GB, 256], F16, tag="ea4", name="ea4")
                    nc.sync.dma_start(
                        ea4[:], ea_hbm.ap()[e0 : e0 + GB * 128, :].rearrange(
                            "(c p) h -> p c h", p=128))
                    r4 = sp.tile([128, GB, 256], dt_h, tag=f"r4{conv}",
                                 name="r4", bufs=(2 if conv == 0 else 1))
                    nc.vector.tensor_tensor(r4[:], hsrc[:], ea4[:],
                                            op=ALU.add)
                    nc.vector.tensor_scalar(r4[:], r4[:], 0.0, None,
                                            op0=ALU.max)
                    # em stays f32r: fp16 exp weights add ~5e-4 relative
                    # noise to every softmax weight, which amplifies through
                    # aggr -> MLP -> instance norm into ~2e-2 output error
                    em = sp.tile([128, GB, 512], F32R, tag="em", name="em",
                                 bufs=1)
                    nc.scalar.activation(em[:, :, 0:256], r4[:], AF.Exp,
                                         scale=tt, bias=ebias[:])
                    nc.vector.tensor_tensor(em[:, :, 256:512], r4[:],
                                            em[:, :, 0:256], op=ALU.mult)
                    s4 = sp.tile([128, GB, 128], F32R, tag="s4", name="s4",
                                 bufs=1)
                    nc.sync.dma_start(
                        s4[:], t_sel.ap()[e0 : e0 + GB * 128, :].rearrange(
                            "(c p) m -> p c m", p=128))
                    for c in range(GB):
                        ci = sub * GB + c
                        nc.tensor.matmul(wp[:], s4[:, c, :], em[:, c, :],
                                         start=(ci == 0), stop=(ci == CPW - 1))
                se = sp2.tile([128, 256], F32, tag="se", name="se", bufs=1)
                nc.vector.tensor_scalar_add(se[:], wp[:, 0:256], 1e-16)
                rs = sp2.tile([128, 256], F32, tag="rs", name="rs", bufs=1)
                nc.vector.reciprocal(rs[:], se[:])
                nc.vector.tensor_tensor(aggr[:, w, :], wp[:, 256:512], rs[:],
                                        op=ALU.mult)

        def node_phase(conv):
            ci = conv
            hin = aggr
            nc.vector.tensor_tensor(hin[:], hpnm[:], aggr[:], op=ALU.add)
            for t0 in range(0, NSLOT_C, 512):
                tw = min(512, NSLOT_C - t0)
                hinT = sp.tile([128, 2, 512], F16, tag="hinT", name="hinT")
                for j in range(tw // 128):
                    w = t0 // 128 + j
                    for cc in range(2):
                        tp = ptp.tile([128, 128], F32, tag="tp", name="tp")
                        nc.tensor.transpose(
                            tp[:], hin[:, w, cc * 128 : (cc + 1) * 128],
                            ident[:])
                        nc.vector.tensor_copy(
                            hinT[:, cc, j * 128 : (j + 1) * 128], tp[:])
                r1 = sp2.tile([128, 4, 512], F16, tag="r1", name="r1", bufs=1)
                for mcp in range(2):
                    m1 = pz.tile([128, 2, 512], F32, tag="zp", name="m1")
                    for half in range(2):
                        mc = mcp * 2 + half
                        for cc in range(2):
                            nc.tensor.matmul(
                                m1[:, half, 0:tw],
                                w1_sb[ci][:, cc * 512 + mc * 128 :
                                          cc * 512 + (mc + 1) * 128],
                                hinT[:, cc, 0:tw],
                                start=(cc == 0), stop=(cc == 1))
                        nc.scalar.activation(
                            r1[:, mc, 0:tw], m1[:, half, 0:tw], AF.Relu,
                            bias=b1_sb[ci][:, mc : mc + 1])
                m2 = pz.tile([128, 2, 512], F32, tag="zp", name="m2")
                for mc in range(2):
                    for kc in range(4):
                        nc.tensor.matmul(
                            m2[:, mc, 0:tw],
                            w2_sb[ci][:, kc * 256 + mc * 128 :
                                      kc * 256 + (mc + 1) * 128],
                            r1[:, kc, 0:tw], start=(kc == 0), stop=(kc == 3))
                    nc.scalar.activation(
                        h2T[mc][:, t0 : t0 + tw], m2[:, mc, 0:tw], AF.Relu,
                        bias=b2_sb[ci][:, mc : mc + 1])
            # instance norm (uniform per-graph slot ranges)
            stat = sp2.tile([128, 4 * GPC], F32, tag="stat", name="stat")
            for cc in range(2):
                for t0 in range(0, NSLOT_C, 512):
                    tw = min(512, NSLOT_C - t0)
                    nc.vector.tensor_tensor(h2T[cc][:, t0 : t0 + tw],
                                            h2T[cc][:, t0 : t0 + tw],
                                            mask_sb[:, t0 : t0 + tw],
                                            op=ALU.mult)
                sqg = sp2.tile([128, S], F32, tag="sqg", name="sqg", bufs=1)
                for g in range(GPC):
                    nc.vector.tensor_reduce(
                        stat[:, cc * GPC + g : cc * GPC + g + 1],
                        h2T[cc][:, g * S : (g + 1) * S], axis=AX.X, op=ALU.add)
                    nc.scalar.activation(
                        sqg[:], h2T[cc][:, g * S : (g + 1) * S], AF.Square,
                        accum_out=stat[:, (2 + cc) * GPC + g :
                                       (2 + cc) * GPC + g + 1])
            mean = sp2.tile([128, 2 * GPC], F32, tag="mean", name="mean")
            nc.vector.tensor_tensor(mean[:], stat[:, 0 : 2 * GPC], ic2_sb[:],
                                    op=ALU.mult)
            var = sp2.tile([128, 2 * GPC], F32, tag="var", name="var")
            nc.vector.tensor_tensor(var[:], stat[:, 2 * GPC : 4 * GPC],
                                    ic2_sb[:], op=ALU.mult)
            msq = sp2.tile([128, 2 * GPC], F32, tag="msq", name="msq")
            nc.vector.tensor_tensor(msq[:], mean[:], mean[:], op=ALU.mult)
            nc.vector.tensor_tensor(var[:], var[:], msq[:], op=ALU.subtract)
            sd = sp2.tile([128, 2 * GPC], F32, tag="sd", name="sd")
            nc.scalar.activation(sd[:], var[:], AF.Sqrt, bias=epsb[:])
            rstd = sp2.tile([128, 2 * GPC], F32, tag="rstd", name="rstd")
            nc.vector.reciprocal(rstd[:], sd[:])
            mr = sp2.tile([128, 2 * GPC], F32, tag="mr", name="mr")
            nc.vector.tensor_tensor(mr[:], mean[:], rstd[:], op=ALU.mult)
            for cc in range(2):
                for g in range(GPC):
                    nc.vector.tensor_scalar(
                        h2T[cc][:, g * S : (g + 1) * S],
                        h2T[cc][:, g * S : (g + 1) * S],
                        rstd[:, cc * GPC + g : cc * GPC + g + 1],
                        mr[:, cc * GPC + g : cc * GPC + g + 1],
                        op0=ALU.mult, op1=ALU.subtract)
                if conv == 0:
                    # zero pad slots: their value feeds conv2's exp via
                    # gathers of pad edges; keep it bounded (and exact 0)
                    nc.vector.tensor_tensor(h2T[cc][:], h2T[cc][:],
                                            mask_sb[:], op=ALU.mult)
                else:
                    # pad slots -> very negative so pooling max ignores them
                    nc.vector.tensor_tensor(h2T[cc][:], h2T[cc][:],
                                            padneg_sb[:], op=ALU.add)

        edge_phase(0)
        node_phase(0)
        for w in range(NW):
            for cc in range(2):
                tp = ptp.tile([128, 128], F32, tag="tp", name="tp")
                nc.tensor.transpose(tp[:],
                                    h2T[cc][:, w * 128 : (w + 1) * 128],
                                    ident[:])
                nc.vector.tensor_copy(hpnm[:, w, cc * 128 : (cc + 1) * 128],
                                      tp[:])
        nc.sync.dma_start(
            agin2.ap().rearrange("(w p) h -> p w h", p=128), hpnm[:])
        nc.gpsimd.collective_compute(
            "AllGather", ALU.bypass, replica_groups=RG,
            ins=[agin2.ap().opt()], outs=[agout2.ap().opt()])

        edge_phase(1)
        node_phase(1)
        pooled = sp2.tile([128, 2, GPC], F32, tag="pooled", name="pooled")
        for cc in range(2):
            for g in range(GPC):
                nc.vector.tensor_reduce(
                    pooled[:, cc, g : g + 1], h2T[cc][:, g * S : (g + 1) * S],
                    axis=AX.X, op=ALU.max)
        fo = pwin.tile([GPC, OUT], F32, tag="fo", name="fo")
        for cc in range(2):
            nc.tensor.matmul(fo[:], pooled[:, cc, :],
                             linw_sb[:, cc * OUT : (cc + 1) * OUT],
                             start=(cc == 0), stop=(cc == 1))
        fs = sp2.tile([GPC, OUT], F32, tag="fs", name="fs")
        nc.vector.tensor_tensor(fs[:], fo[:], linb_sb[:], op=ALU.add)
        fsig = sp2.tile([GPC, OUT], F32, tag="fsig", name="fsig")
        nc.scalar.activation(fsig[:], fs[:], AF.Sigmoid)
        nc.sync.dma_start(agin3.ap(), fsig[:])
        nc.gpsimd.collective_compute(
            "AllGather", ALU.bypass, replica_groups=RG,
            ins=[agin3.ap().opt()], outs=[agout3.ap().opt()])
        ofin = sp2.tile([G, OUT], F32, tag="ofin", name="ofin")
        nc.sync.dma_start(ofin[:], agout3.ap())
        nc.sync.dma_start(t_out.ap(), ofin[:])

    nc.compile()
    return nc


def _in_maps(d):
    in_maps = []
    for k in range(NCORE):
        in_maps.append(dict(
            x_loc=d["x_loc"][k], eaT=d["eaT"][k], sel=d["sel"][k],
            idxs=d["idxs"][k], icnt2=d["icnt2"][k], mask=d["mask"][k],
            padneg=d["padneg"][k], nodeW=d["nodeW"], edgeW=d["edgeW"],
            w1_1=d["w1_1"], b1_1=d["b1_1"], w2_1=d["w2_1"], b2_1=d["b2_1"],
            w1_2=d["w1_2"], b1_2=d["b1_2"], w2_2=d["w2_2"], b2_2=d["b2_2"],
            linw=d["linw"], linb=d["linb"]))
    return in_maps


def kernel(**inputs):
    global LAST_EXEC_NS, LAST_RES
    d = _prep(inputs)
    key = (d["S"], d["CPW"])
    if key not in _CACHE:
        _CACHE[key] = _build(d)
    nc = _CACHE[key]
    res = run_bass_kernel_spmd(nc, _in_maps(d), core_ids=list(range(NCORE)))
    LAST_EXEC_NS = res.exec_time_ns
    LAST_RES = res
    return res.results[0]["out"]
